# revision 2
# baseline (speedup 1.0000x reference)
"""GCNConv Trainium2 kernel, 8-core SPMD, fp8-DoubleRow aggregation.

Math: out = D^-1/2 A D^-1/2 (x W^T + b), A = adjacency (+self loops,
duplicate edges collapse to 1).

Aggregate-first (no cross-core traffic):
    s    = deg^-1/2                          (host)
    xt   = s (.) x                           (host)
    agg  = A @ xt                            (device matmul 1, row-sharded)
    aggs = A @ s                             (host, feeds bias term)
    out  = s (.) ([agg, aggs] @ [W^T; b])    (device matmul 2 + fused scale)

Matmul 1 runs at 2x fp16 rate via fp8e4 MatmulPerfMode.DoubleRow: each
instruction contracts a PAIR of 128-row k-subtiles (lhsT [128,2,128],
rhs [128,2,512] -> psum [128,512]) in the same ~213ns a single fp16
128-contraction instruction takes.  xt in fp8e4 alone gives rel_l2
~2.26e-2 (> the 2e-2 gate), so a partial low-order correction is added:
xt ~= hi + lo (both fp8e4), and the lo term is applied for the NLO/32
fraction of source rows with the largest residual energy x out-degree.
The host permutes the contraction (source) order so those rows land in
dedicated pair-tiles, which share the streamed A tiles with the hi pass.
NLO=16 measures rel_l2 ~1.4e-2.

Matmul 2 (agg @ W^T + bias, fp16) is unchanged from the fp16 version.

Full inputs in, full outputs out; sharding is internal (each core gets
its own A^T column slice / aggs slice / s slice; hi, lo, W, b broadcast).
"""

import functools
import numpy as np

N = 8192
D = 512
NCORES = 8
ROWS = N // NCORES          # 1024 output rows per core
P = 128
KP = N // (2 * P)           # 32 contraction pair-tiles (256 rows each)
FT = D // P                 # 4 feature tiles
NH = ROWS // D              # 2 dst halves of 512 per core
MT = ROWS // P              # 8 output row chunks per core
NLO = 16                    # pair-tiles receiving the lo correction (<=16)

_HALF = "float16"


def _corr(kp, nlo=NLO):
    """Corrected pair-tiles sit at even slots 0,2,..,2(nlo-1)."""
    return kp % 2 == 0 and kp // 2 < nlo


def _kernel_body(tc, aps, bufs=6, const_after_k=1, nlo=NLO):
    import concourse.mybir as mybir

    nc = tc.nc
    at, hi, lo, wt, brow, aggs, sc, out = (
        aps["at"], aps["hi"], aps["lo"], aps["wt"], aps["brow"],
        aps["aggs"], aps["sc"], aps["out"],
    )
    half = mybir.dt.float16 if _HALF == "float16" else mybir.dt.bfloat16
    fp8 = mybir.dt.float8e4
    f32 = mybir.dt.float32
    DR = mybir.MatmulPerfMode.DoubleRow

    with (
        tc.tile_pool(name="hi_pool", bufs=bufs) as hi_pool,
        tc.tile_pool(name="lo_pool", bufs=max(bufs // 2, 2)) as lo_pool,
        tc.tile_pool(name="at_pool", bufs=bufs) as at_pool,
        tc.tile_pool(name="psum", bufs=1, space="PSUM") as psum_pool,
        tc.tile_pool(name="aggT_pool", bufs=NH * FT) as aggT_pool,
        tc.tile_pool(name="out_pool", bufs=3) as out_pool,
        tc.tile_pool(name="const", bufs=1) as const,
    ):
        wt_sb = []
        b_sb = aggs_sb = s_sb = None

        def emit_consts():
            nonlocal b_sb, aggs_sb, s_sb
            for i in range(FT):
                w_t = const.tile([P, D], half, tag="wt", bufs=FT,
                                 name=f"wt{i}")
                nc.sync.dma_start(out=w_t[:], in_=wt[i * P:(i + 1) * P, :])
                wt_sb.append(w_t)
            b_sb = const.tile([1, D], half, tag="b", name="b_sb")
            nc.sync.dma_start(out=b_sb[:], in_=brow[:])
            aggs_sb = const.tile([1, ROWS], half, tag="aggs", name="aggs_sb")
            nc.sync.dma_start(out=aggs_sb[:], in_=aggs[:])
            s_sb = const.tile([P, MT], f32, tag="s", name="s_sb")
            nc.sync.dma_start(out=s_sb[:], in_=sc[:])

        if const_after_k is None:
            emit_consts()

        # ---- matmul 1: aggT[f, dst] += hi/lo[k-pair].T @ at[k-pair] ----
        # psum[f] = [128 f-rows, 1024 dst] fp32 (2 banks); 4 tiles = 8 banks
        psum = []
        for f in range(FT):
            ps = psum_pool.tile([P, ROWS], f32, tag=f"ps{f}", name=f"ps{f}")
            psum.append(ps)
        nlo_seen = 0
        for kp in range(KP):
            hi_t = hi_pool.tile([P, 2, D], fp8, tag="hi", name=f"hi{kp}")
            nc.sync.dma_start(out=hi_t[:], in_=hi[kp * P:(kp + 1) * P, :])
            at_t = at_pool.tile([P, 2, ROWS], fp8, tag="at", name=f"at{kp}")
            nc.sync.dma_start(out=at_t[:], in_=at[kp * P:(kp + 1) * P, :])
            lo_t = None
            if _corr(kp, nlo):
                j = nlo_seen
                nlo_seen += 1
                lo_t = lo_pool.tile([P, 2, D], fp8, tag="lo", name=f"lo{kp}")
                nc.sync.dma_start(out=lo_t[:], in_=lo[j * P:(j + 1) * P, :])
            if const_after_k == kp:
                emit_consts()
            last = kp == KP - 1
            for f in range(FT):
                for h in range(NH):
                    nc.tensor.matmul(
                        psum[f][:, h * D:(h + 1) * D],
                        hi_t[:, :, f * P:(f + 1) * P],
                        at_t[:, :, h * D:(h + 1) * D],
                        start=(kp == 0), stop=last, perf_mode=DR,
                    )
                    if lo_t is not None:
                        nc.tensor.matmul(
                            psum[f][:, h * D:(h + 1) * D],
                            lo_t[:, :, f * P:(f + 1) * P],
                            at_t[:, :, h * D:(h + 1) * D],
                            start=False, stop=False, perf_mode=DR,
                        )

        # evict (fp32 -> fp16 cast); aggT[n*FT+f] is [128 f, 512 dst-half n]
        aggT = [None] * (NH * FT)
        for f in range(FT):
            for n in range(NH):
                agg_t = aggT_pool.tile([P, D], half, tag="aggT",
                                       name=f"aggT{n}_{f}")
                nc.vector.tensor_copy(agg_t[:], psum[f][:, n * D:(n + 1) * D])
                aggT[n * FT + f] = agg_t

        # ---- matmul 2 + fused s-scale on eviction ----
        for m in range(MT):
            n, off = m // FT, (m % FT) * P
            ps2 = psum_pool.tile([P, D], f32, tag=f"ps{m % 2}",
                                 name=f"ps2_{m}")
            for kf in range(FT):
                nc.tensor.matmul(
                    ps2[:],
                    aggT[n * FT + kf][:, off:off + P],
                    wt_sb[kf][:],
                    start=(kf == 0),
                    stop=False,
                )
            nc.tensor.matmul(
                ps2[:],
                aggs_sb[:, m * P:(m + 1) * P],
                b_sb[:],
                start=False,
                stop=True,
            )
            o_t = out_pool.tile([P, D], f32, tag="o", name=f"o{m}")
            nc.scalar.activation(
                o_t[:], ps2[:], mybir.ActivationFunctionType.Copy,
                scale=s_sb[:, m:m + 1],
            )
            nc.sync.dma_start(out=out[m * P:(m + 1) * P, :], in_=o_t[:])


@functools.lru_cache(maxsize=8)
def _build(repeat=1, bufs=6, const_after_k=1, nlo=NLO):
    import concourse.bacc as bacc
    import concourse.mybir as mybir
    import concourse.tile as tile

    half = mybir.dt.float16 if _HALF == "float16" else mybir.dt.bfloat16
    fp8 = mybir.dt.float8e4
    nc = bacc.Bacc("TRN2", target_bir_lowering=False, debug=False,
                   num_devices=NCORES)
    aps = {
        "at": nc.dram_tensor("at", [KP * P, 2 * ROWS], fp8,
                             kind="ExternalInput").ap(),
        "hi": nc.dram_tensor("hi", [KP * P, 2 * D], fp8,
                             kind="ExternalInput").ap(),
        "lo": nc.dram_tensor("lo", [max(NLO, 1) * P, 2 * D], fp8,
                             kind="ExternalInput").ap(),
        "wt": nc.dram_tensor("wt", [D, D], half, kind="ExternalInput").ap(),
        "brow": nc.dram_tensor("brow", [1, D], half, kind="ExternalInput").ap(),
        "aggs": nc.dram_tensor("aggs", [1, ROWS], half,
                               kind="ExternalInput").ap(),
        "sc": nc.dram_tensor("sc", [P, MT], mybir.dt.float32,
                             kind="ExternalInput").ap(),
        "out": nc.dram_tensor("out", [ROWS, D], mybir.dt.float32,
                              kind="ExternalOutput").ap(),
    }
    with tile.TileContext(nc) as tc:
        for _ in range(repeat):
            _kernel_body(tc, aps, bufs=bufs, const_after_k=const_after_k,
                         nlo=nlo)
    nc.compile()
    return nc


def _pair_pack(v):
    """[8192, W] -> [4096, 2*W]: row (kp*128+p), pair slot-major cols."""
    W = v.shape[1]
    return np.ascontiguousarray(
        v.reshape(KP, 2, P, W).transpose(0, 2, 1, 3).reshape(KP * P, 2 * W))


def _prep(x, edge_index, W, b, nlo=NLO):
    """Host-side scatter/quantize/permute; returns per-core input maps."""
    import ml_dtypes
    half = np.dtype(_HALF)
    fp8 = ml_dtypes.float8_e4m3
    ei = np.asarray(edge_index)
    # AT[j, r] = A[r, j]; duplicates collapse via assignment, + self loops
    AT = np.zeros((N, N), dtype=np.uint8)
    AT[ei[1].astype(np.int64), ei[0].astype(np.int64)] = 1
    idx = np.arange(N)
    AT[idx, idx] = 1
    deg = AT.sum(axis=0, dtype=np.int64).astype(np.float64)  # A row sums
    s = (1.0 / np.sqrt(deg)).astype(np.float32)
    aggs = (AT.T.astype(np.float32) @ s).astype(half)        # A @ s
    xt = (s[:, None] * np.asarray(x)).astype(np.float32)
    hi = xt.astype(fp8)
    lo32 = xt - hi.astype(np.float32)
    lo = lo32.astype(fp8)
    wt = np.ascontiguousarray(np.asarray(W).T).astype(half)
    brow = np.asarray(b).reshape(1, D).astype(half)

    # permute sources: high residual-importance rows into corrected slots
    imp = (lo32.astype(np.float64) ** 2).sum(axis=1) * AT.sum(
        axis=1, dtype=np.int64)
    order = np.argsort(-imp)
    corr_slots = [kp for kp in range(KP) if _corr(kp, nlo)]
    rest_slots = [kp for kp in range(KP) if not _corr(kp, nlo)]
    perm = np.empty(N, dtype=np.int64)
    pos = 0
    for kp in corr_slots:
        perm[kp * 2 * P:(kp + 1) * 2 * P] = order[pos:pos + 2 * P]
        pos += 2 * P
    for kp in rest_slots:
        perm[kp * 2 * P:(kp + 1) * 2 * P] = order[pos:pos + 2 * P]
        pos += 2 * P

    at_perm = AT[perm]                       # [src-slot, dst]
    hi_p = _pair_pack(hi[perm]).astype(fp8)
    lo_all = lo[perm].reshape(KP, 2, P, D).transpose(0, 2, 1, 3)
    lo_p = np.ascontiguousarray(
        lo_all[corr_slots].reshape(len(corr_slots) * P, 2 * D)).astype(fp8)
    if len(corr_slots) == 0:
        lo_p = np.zeros((P, 2 * D), dtype=fp8)

    in_maps = []
    for c in range(NCORES):
        cols = slice(c * ROWS, (c + 1) * ROWS)
        rows = slice(c * ROWS, (c + 1) * ROWS)
        at_c = _pair_pack(np.ascontiguousarray(at_perm[:, cols])).astype(fp8)
        in_maps.append({
            "at": at_c,
            "hi": hi_p,
            "lo": lo_p,
            "wt": wt,
            "brow": brow,
            "aggs": np.ascontiguousarray(aggs[rows]).reshape(1, ROWS),
            # sc[p, m] = s[c*1024 + m*128 + p]
            "sc": np.ascontiguousarray(
                s[rows].reshape(MT, P).T).astype(np.float32),
        })
    return in_maps


def kernel(x, edge_index, W, b):
    import time
    from concourse import bass_utils

    nc = _build()
    in_maps = _prep(x, edge_index, W, b)
    last = None
    for attempt in range(3):
        try:
            res = bass_utils.run_bass_kernel_spmd(
                nc, in_maps, core_ids=list(range(NCORES)))
            return np.concatenate(
                [res.results[c]["out"] for c in range(NCORES)], axis=0)
        except Exception as e:  # transient NRT device flakes recover on retry
            last = e
            time.sleep(5.0)
    raise last


# revision 4
# speedup vs baseline: 8.5416x; 8.5416x over previous
"""GCNConv Trainium2 kernel, 8-core SPMD, fp8-DoubleRow aggregation.

Math: out = D^-1/2 A D^-1/2 (x W^T + b), A = adjacency (+self loops,
duplicate edges collapse to 1).

Aggregate-first (no cross-core traffic):
    s    = deg^-1/2                          (host)
    xt   = s (.) x                           (host)
    agg  = A @ xt                            (device matmul 1, row-sharded)
    aggs = A @ s                             (host, feeds bias term)
    out  = s (.) ([agg, aggs] @ [W^T; b])    (device matmul 2 + fused scale)

Matmul 1 runs at 2x fp16 rate via fp8e4 MatmulPerfMode.DoubleRow: each
instruction contracts a PAIR of 128-row k-subtiles (lhsT [128,2,128],
rhs [128,2,512] -> psum [128,512]) in the same ~213ns a single fp16
128-contraction instruction takes.  xt in fp8e4 alone gives rel_l2
~2.26e-2 (> the 2e-2 gate), so a partial low-order correction is added:
xt ~= hi + lo (both fp8e4), and the lo term is applied for the NLO/32
fraction of source rows with the largest residual energy x out-degree.
The host permutes the contraction (source) order so those rows land in
dedicated pair-tiles, which share the streamed A tiles with the hi pass.
NLO=16 measures rel_l2 ~1.4e-2.

Matmul 2 (agg @ W^T + bias, fp16) is unchanged from the fp16 version.

Full inputs in, full outputs out; sharding is internal (each core gets
its own A^T column slice / aggs slice / s slice; hi, lo, W, b broadcast).
"""

import functools
import numpy as np

N = 8192
D = 512
NCORES = 8
ROWS = N // NCORES          # 1024 output rows per core
P = 128
KP = N // (2 * P)           # 32 contraction pair-tiles (256 rows each)
FT = D // P                 # 4 feature tiles
NH = ROWS // D              # 2 dst halves of 512 per core
MT = ROWS // P              # 8 output row chunks per core
NLO = 12                    # pair-tiles receiving the lo correction (<=16)

_HALF = "float16"


def _corr(kp, nlo=NLO):
    """Corrected pair-tiles sit at even slots 0,2,..,2(nlo-1)."""
    return kp % 2 == 0 and kp // 2 < nlo


def _kernel_body(tc, aps, bufs=6, const_after_k=1, nlo=NLO):
    import concourse.mybir as mybir

    nc = tc.nc
    at, hi, lo, wt, brow, aggs, sc, out = (
        aps["at"], aps["hi"], aps["lo"], aps["wt"], aps["brow"],
        aps["aggs"], aps["sc"], aps["out"],
    )
    half = mybir.dt.float16 if _HALF == "float16" else mybir.dt.bfloat16
    fp8 = mybir.dt.float8e4
    f32 = mybir.dt.float32
    DR = mybir.MatmulPerfMode.DoubleRow

    with (
        tc.tile_pool(name="hi_pool", bufs=bufs) as hi_pool,
        tc.tile_pool(name="lo_pool", bufs=max(bufs // 2, 2)) as lo_pool,
        tc.tile_pool(name="at_pool", bufs=bufs) as at_pool,
        tc.tile_pool(name="psum", bufs=1, space="PSUM") as psum_pool,
        tc.tile_pool(name="aggT_pool", bufs=NH * FT) as aggT_pool,
        tc.tile_pool(name="out_pool", bufs=3) as out_pool,
        tc.tile_pool(name="const", bufs=1) as const,
    ):
        wt_sb = []
        b_sb = aggs_sb = s_sb = None

        def emit_consts():
            nonlocal b_sb, aggs_sb, s_sb
            for i in range(FT):
                w_t = const.tile([P, D], half, tag="wt", bufs=FT,
                                 name=f"wt{i}")
                nc.sync.dma_start(out=w_t[:], in_=wt[i * P:(i + 1) * P, :])
                wt_sb.append(w_t)
            b_sb = const.tile([1, D], half, tag="b", name="b_sb")
            nc.sync.dma_start(out=b_sb[:], in_=brow[:])
            aggs_sb = const.tile([1, ROWS], half, tag="aggs", name="aggs_sb")
            nc.sync.dma_start(out=aggs_sb[:], in_=aggs[:])
            s_sb = const.tile([P, MT], f32, tag="s", name="s_sb")
            nc.sync.dma_start(out=s_sb[:], in_=sc[:])

        if const_after_k is None:
            emit_consts()

        # ---- matmul 1: aggT[f, dst] += hi/lo[k-pair].T @ at[k-pair] ----
        # psum[f] = [128 f-rows, 1024 dst] fp32 (2 banks); 4 tiles = 8 banks
        psum = []
        for f in range(FT):
            ps = psum_pool.tile([P, ROWS], f32, tag=f"ps{f}", name=f"ps{f}")
            psum.append(ps)
        nlo_seen = 0
        for kp in range(KP):
            hi_t = hi_pool.tile([P, 2, D], fp8, tag="hi", name=f"hi{kp}")
            nc.sync.dma_start(out=hi_t[:], in_=hi[kp * P:(kp + 1) * P, :])
            at_t = at_pool.tile([P, 2, ROWS], fp8, tag="at", name=f"at{kp}")
            nc.sync.dma_start(out=at_t[:], in_=at[kp * P:(kp + 1) * P, :])
            lo_t = None
            if _corr(kp, nlo):
                j = nlo_seen
                nlo_seen += 1
                lo_t = lo_pool.tile([P, 2, D], fp8, tag="lo", name=f"lo{kp}")
                nc.sync.dma_start(out=lo_t[:], in_=lo[j * P:(j + 1) * P, :])
            if const_after_k == kp:
                emit_consts()
            last = kp == KP - 1
            for f in range(FT):
                for h in range(NH):
                    nc.tensor.matmul(
                        psum[f][:, h * D:(h + 1) * D],
                        hi_t[:, :, f * P:(f + 1) * P],
                        at_t[:, :, h * D:(h + 1) * D],
                        start=(kp == 0), stop=last, perf_mode=DR,
                    )
                    if lo_t is not None:
                        nc.tensor.matmul(
                            psum[f][:, h * D:(h + 1) * D],
                            lo_t[:, :, f * P:(f + 1) * P],
                            at_t[:, :, h * D:(h + 1) * D],
                            start=False, stop=False, perf_mode=DR,
                        )

        # evict (fp32 -> fp16 cast); aggT[n*FT+f] is [128 f, 512 dst-half n]
        aggT = [None] * (NH * FT)
        for f in range(FT):
            for n in range(NH):
                agg_t = aggT_pool.tile([P, D], half, tag="aggT",
                                       name=f"aggT{n}_{f}")
                nc.vector.tensor_copy(agg_t[:], psum[f][:, n * D:(n + 1) * D])
                aggT[n * FT + f] = agg_t

        # ---- matmul 2 + fused s-scale on eviction ----
        for m in range(MT):
            n, off = m // FT, (m % FT) * P
            ps2 = psum_pool.tile([P, D], f32, tag=f"ps{m % 2}",
                                 name=f"ps2_{m}")
            for kf in range(FT):
                nc.tensor.matmul(
                    ps2[:],
                    aggT[n * FT + kf][:, off:off + P],
                    wt_sb[kf][:],
                    start=(kf == 0),
                    stop=False,
                )
            nc.tensor.matmul(
                ps2[:],
                aggs_sb[:, m * P:(m + 1) * P],
                b_sb[:],
                start=False,
                stop=True,
            )
            o_t = out_pool.tile([P, D], f32, tag="o", name=f"o{m}")
            nc.scalar.activation(
                o_t[:], ps2[:], mybir.ActivationFunctionType.Copy,
                scale=s_sb[:, m:m + 1],
            )
            nc.sync.dma_start(out=out[m * P:(m + 1) * P, :], in_=o_t[:])


@functools.lru_cache(maxsize=8)
def _build_loop(L, bufs=6, const_after_k=1, nlo=NLO):
    """Body wrapped in a hardware For_i loop: L bodies per NEFF call.

    For low-noise device timing only (the back-edge barrier adds a few
    us/iter of pessimism vs the unrolled steady state).
    """
    import concourse.bacc as bacc
    import concourse.mybir as mybir
    import concourse.tile as tile

    half = mybir.dt.float16 if _HALF == "float16" else mybir.dt.bfloat16
    fp8 = mybir.dt.float8e4
    nc = bacc.Bacc("TRN2", target_bir_lowering=False, debug=False,
                   num_devices=NCORES)
    aps = {
        "at": nc.dram_tensor("at", [KP * P, 2 * ROWS], fp8,
                             kind="ExternalInput").ap(),
        "hi": nc.dram_tensor("hi", [KP * P, 2 * D], fp8,
                             kind="ExternalInput").ap(),
        "lo": nc.dram_tensor("lo", [max(NLO, 1) * P, 2 * D], fp8,
                             kind="ExternalInput").ap(),
        "wt": nc.dram_tensor("wt", [D, D], half, kind="ExternalInput").ap(),
        "brow": nc.dram_tensor("brow", [1, D], half, kind="ExternalInput").ap(),
        "aggs": nc.dram_tensor("aggs", [1, ROWS], half,
                               kind="ExternalInput").ap(),
        "sc": nc.dram_tensor("sc", [P, MT], mybir.dt.float32,
                             kind="ExternalInput").ap(),
        "out": nc.dram_tensor("out", [ROWS, D], mybir.dt.float32,
                              kind="ExternalOutput").ap(),
    }
    with tile.TileContext(nc) as tc:
        with tc.For_i(0, L, 1):
            _kernel_body(tc, aps, bufs=bufs, const_after_k=const_after_k,
                         nlo=nlo)
    nc.compile()
    return nc


@functools.lru_cache(maxsize=8)
def _build(repeat=1, bufs=6, const_after_k=1, nlo=NLO):
    import concourse.bacc as bacc
    import concourse.mybir as mybir
    import concourse.tile as tile

    half = mybir.dt.float16 if _HALF == "float16" else mybir.dt.bfloat16
    fp8 = mybir.dt.float8e4
    nc = bacc.Bacc("TRN2", target_bir_lowering=False, debug=False,
                   num_devices=NCORES)
    aps = {
        "at": nc.dram_tensor("at", [KP * P, 2 * ROWS], fp8,
                             kind="ExternalInput").ap(),
        "hi": nc.dram_tensor("hi", [KP * P, 2 * D], fp8,
                             kind="ExternalInput").ap(),
        "lo": nc.dram_tensor("lo", [max(NLO, 1) * P, 2 * D], fp8,
                             kind="ExternalInput").ap(),
        "wt": nc.dram_tensor("wt", [D, D], half, kind="ExternalInput").ap(),
        "brow": nc.dram_tensor("brow", [1, D], half, kind="ExternalInput").ap(),
        "aggs": nc.dram_tensor("aggs", [1, ROWS], half,
                               kind="ExternalInput").ap(),
        "sc": nc.dram_tensor("sc", [P, MT], mybir.dt.float32,
                             kind="ExternalInput").ap(),
        "out": nc.dram_tensor("out", [ROWS, D], mybir.dt.float32,
                              kind="ExternalOutput").ap(),
    }
    with tile.TileContext(nc) as tc:
        for _ in range(repeat):
            _kernel_body(tc, aps, bufs=bufs, const_after_k=const_after_k,
                         nlo=nlo)
    nc.compile()
    return nc


def _pair_pack(v):
    """[8192, W] -> [4096, 2*W]: row (kp*128+p), pair slot-major cols."""
    W = v.shape[1]
    return np.ascontiguousarray(
        v.reshape(KP, 2, P, W).transpose(0, 2, 1, 3).reshape(KP * P, 2 * W))


def _prep(x, edge_index, W, b, nlo=NLO):
    """Host-side scatter/quantize/permute; returns per-core input maps."""
    import ml_dtypes
    half = np.dtype(_HALF)
    fp8 = ml_dtypes.float8_e4m3
    ei = np.asarray(edge_index)
    # AT[j, r] = A[r, j]; duplicates collapse via assignment, + self loops
    AT = np.zeros((N, N), dtype=np.uint8)
    AT[ei[1].astype(np.int64), ei[0].astype(np.int64)] = 1
    idx = np.arange(N)
    AT[idx, idx] = 1
    deg = AT.sum(axis=0, dtype=np.int64).astype(np.float64)  # A row sums
    s = (1.0 / np.sqrt(deg)).astype(np.float32)
    aggs = (AT.T.astype(np.float32) @ s).astype(half)        # A @ s
    xt = (s[:, None] * np.asarray(x)).astype(np.float32)
    hi = xt.astype(fp8)
    lo32 = xt - hi.astype(np.float32)
    lo = lo32.astype(fp8)
    wt = np.ascontiguousarray(np.asarray(W).T).astype(half)
    brow = np.asarray(b).reshape(1, D).astype(half)

    # permute sources: high residual-importance rows into corrected slots
    imp = (lo32.astype(np.float64) ** 2).sum(axis=1) * AT.sum(
        axis=1, dtype=np.int64)
    order = np.argsort(-imp)
    corr_slots = [kp for kp in range(KP) if _corr(kp, nlo)]
    rest_slots = [kp for kp in range(KP) if not _corr(kp, nlo)]
    perm = np.empty(N, dtype=np.int64)
    pos = 0
    for kp in corr_slots:
        perm[kp * 2 * P:(kp + 1) * 2 * P] = order[pos:pos + 2 * P]
        pos += 2 * P
    for kp in rest_slots:
        perm[kp * 2 * P:(kp + 1) * 2 * P] = order[pos:pos + 2 * P]
        pos += 2 * P

    at_perm = AT[perm]                       # [src-slot, dst]
    hi_p = _pair_pack(hi[perm]).astype(fp8)
    lo_all = lo[perm].reshape(KP, 2, P, D).transpose(0, 2, 1, 3)
    lo_p = np.ascontiguousarray(
        lo_all[corr_slots].reshape(len(corr_slots) * P, 2 * D)).astype(fp8)
    if len(corr_slots) == 0:
        lo_p = np.zeros((P, 2 * D), dtype=fp8)

    in_maps = []
    for c in range(NCORES):
        cols = slice(c * ROWS, (c + 1) * ROWS)
        rows = slice(c * ROWS, (c + 1) * ROWS)
        at_c = _pair_pack(np.ascontiguousarray(at_perm[:, cols])).astype(fp8)
        in_maps.append({
            "at": at_c,
            "hi": hi_p,
            "lo": lo_p,
            "wt": wt,
            "brow": brow,
            "aggs": np.ascontiguousarray(aggs[rows]).reshape(1, ROWS),
            # sc[p, m] = s[c*1024 + m*128 + p]
            "sc": np.ascontiguousarray(
                s[rows].reshape(MT, P).T).astype(np.float32),
        })
    return in_maps


def kernel(x, edge_index, W, b):
    import time
    from concourse import bass_utils

    nc = _build()
    in_maps = _prep(x, edge_index, W, b)
    last = None
    for attempt in range(3):
        try:
            res = bass_utils.run_bass_kernel_spmd(
                nc, in_maps, core_ids=list(range(NCORES)))
            return np.concatenate(
                [res.results[c]["out"] for c in range(NCORES)], axis=0)
        except Exception as e:  # transient NRT device flakes recover on retry
            last = e
            time.sleep(5.0)
    raise last
